# revision 2
# baseline (speedup 1.0000x reference)
"""CausalShapedAttention Trainium2 Bass kernel, v2.

y = (beta*softmax(causal(q k^T / sqrt(D))) + alpha*I - gamma*MC) @ v
  qk = x @ w_attn^T (q,k halves), v = x reshaped; B=2, T=2048, C=1024,
  H=16, D=64.  MC[i,j] = 1/(T-1-i) for j>i (i<T-1); MC[T-1,:] = 1/T.

Sharding: 8 cores; core c -> batch b=c//4, head-group g=c%4 (4 heads).
Fully independent cores, no collectives.

Design (cost model: matmul cost = out-free-size * cyc/row; bf16 1x,
fp8e4 DoubleRow 0.5x with 2 packed K-planes; 2-byte matmuls lower to
Ldweights+Matmult pairs, so PE wait-queue pressure matters):
  - projection: fp8 DoubleRow (x*16, w*16; exp scale absorbs 1/65536),
    out [64,256] at partition 0 (walrus rejects DR outs at base 64);
    per-(tt,mt) waves spread through the chunk stream, 2 psum slots.
  - ST per 128-key block: bf16 k^T slice stationary, q^T moving, PSUM
    [128 keys, n]; tri-mask via bf16 negmask matmul.  Diag blocks packed
    2-per-psum-tile so exp batches: pairs [128,1024]/[128,896]/[128,384].
  - exp on ACT over multi-bank PSUM APs -> bf16 SBUF ex tiles; exact
    causal row count (69632 rows/core; ACT is the wall ~74us).
  - PV flipped: stationary = ex [128k,128q] slice, moving = [v|1/beta]
    bf16 [128,65] -> natural-layout out [128 q, 65] PSUM-accumulated
    over key blocks; sums ride in col 64; per-qb combine = reciprocal +
    scalar_tensor_tensor on DVE.  No transposes, no broadcast matmuls.
    PV groups trail their exp by 2 ST-groups (4-slot PE wait queue).
  - MC + alpha*I in natural layout; mcdiagT/crow/suffix-sums are host
    precomputed (bf16) and DMA'd; per block: rank-1 crow x S_after +
    strict-upper mcdiagT @ v matmuls into one mc psum bank; dense last
    MC row handled by crow15 slot + S_after[15] = total.
  - PSUM banks: big (ST) 2x2, yacc 2, mc/proj-alt 1, proj 1 = 8.
"""
import sys

for _p in ("/opt/trn_rl_repo",):
    if _p not in sys.path:
        sys.path.insert(0, _p)

from contextlib import ExitStack

import numpy as np
import ml_dtypes

import concourse.bass as bass
import concourse.tile as tile
from concourse import bacc, mybir
from concourse.bass_utils import run_bass_kernel_spmd

F32 = mybir.dt.float32
BF16 = mybir.dt.bfloat16
F8 = mybir.dt.float8e4
EXP = mybir.ActivationFunctionType.Exp
OP = mybir.AluOpType
DR = mybir.MatmulPerfMode.DoubleRow
NPF8 = ml_dtypes.float8_e4m3
NPBF = ml_dtypes.bfloat16

B, T, C, H, D = 2, 2048, 1024, 16, 64
HL = 4            # heads per core
GC = HL * D       # channels per head-group (256)
NCORES = 8
NB = T // 128     # 16 key/query row blocks
SX = 16.0         # fp8 input scale for x
SW = 16.0         # fp8 input scale for w

CFG = dict(proj="dr8")
WCOL = {0: 0, 1: 1, 4: 2, 5: 3, 2: 4, 3: 5, 6: 6, 7: 7}  # wt8 col order

LAST_RESULTS = None


def _emit(tc: tile.TileContext, xt8, wt8, xv16, mcdTd, crowd,
          safterd, y, alpha, beta, gamma, cfg):
    nc = tc.nc
    st_scale = 1.0 / (8.0 * SX * SX * SW * SW)

    with ExitStack() as ctx:
        ctx.enter_context(nc.allow_low_precision(
            reason="bf16/fp8 matmuls with fp32 psum accumulation"))
        consts = ctx.enter_context(tc.tile_pool(name="consts", bufs=1))

        # ---------------- psum pools (4 + 2 + 1 + 1 = 8 banks) ------------
        bigp = ctx.enter_context(tc.tile_pool(name="bigp", bufs=2,
                                              space="PSUM"))
        yaccp = ctx.enter_context(tc.tile_pool(name="yaccp", bufs=2,
                                               space="PSUM"))
        mcp = ctx.enter_context(tc.tile_pool(name="mcp", bufs=1,
                                             space="PSUM"))
        projp = ctx.enter_context(tc.tile_pool(name="projp", bufs=1,
                                               space="PSUM"))

        # ---------------- constants -----------------
        identf = consts.tile([128, 128], F32, name="identf", tag="identf")
        nc.vector.memset(identf, 1.0)
        nc.gpsimd.affine_select(
            out=identf, in_=identf, compare_op=OP.is_equal, fill=0.0,
            base=0, pattern=[[-1, 128]], channel_multiplier=1,
        )
        identr = consts.tile([128, 128], BF16, name="identr", tag="identr")
        nc.vector.tensor_copy(out=identr, in_=identf)

        # negmaskT[p, f] = -1e30 where p < f (key p, query f)
        negf = consts.tile([128, 128], F32, name="negf", tag="negf")
        nc.vector.memset(negf, 0.0)
        nc.gpsimd.affine_select(
            out=negf, in_=negf, compare_op=OP.is_ge, fill=-1e30,
            base=0, pattern=[[-1, 128]], channel_multiplier=1,
        )
        negmaskT = consts.tile([128, 128], BF16, name="negmaskT",
                               tag="negmaskT")
        nc.vector.tensor_copy(out=negmaskT, in_=negf)

        # input tiles
        xt8s = consts.tile(list(xt8.shape), xt8.dtype, name="xt8s", tag="xt8s")
        wt8s = consts.tile(list(wt8.shape), wt8.dtype, name="wt8s", tag="wt8s")
        xv = [consts.tile([128, HL * 65], BF16, name=f"xv{bt}", tag=f"xv{bt}")
              for bt in range(NB)]

        # host-precomputed MC constants (bf16); crow/saft rows live on
        # partitions {0,32,64} (row (bi//6)*32, col (bi%6)*width); the
        # bi=15 slots hold the dense-last-row coefs / the total row sum.
        mcdTs = consts.tile([128, NB * 128], BF16, name="mcdTs", tag="mcdTs")
        crow = consts.tile([128, 768], BF16, name="crow", tag="crow")
        saft = consts.tile([128, 1536], BF16, name="saft", tag="saft")

        nc.sync.dma_start(out=wt8s, in_=wt8)
        ncols = xt8.shape[1] // 8
        for tt in range(2):
            nc.sync.dma_start(out=xt8s[:, tt * ncols:(tt + 1) * ncols],
                              in_=xt8[:, tt * ncols:(tt + 1) * ncols])
        for bt in range(4):
            nc.sync.dma_start(out=xv[bt],
                              in_=xv16[bt * 128:(bt + 1) * 128, :])
        nc.sync.dma_start(out=crow, in_=crowd)
        nc.sync.dma_start(out=saft, in_=safterd)
        nc.sync.dma_start(out=mcdTs, in_=mcdTd)
        for bt in range(4, NB):
            nc.sync.dma_start(out=xv[bt],
                              in_=xv16[bt * 128:(bt + 1) * 128, :])
        for tt in range(2, 8):
            nc.sync.dma_start(out=xt8s[:, tt * ncols:(tt + 1) * ncols],
                              in_=xt8[:, tt * ncols:(tt + 1) * ncols])

        # persistent SBUF
        qkT = [consts.tile([128, T], BF16, name=f"qkT{mt}", tag=f"qkT{mt}")
               for mt in range(4)]
        mcnat = [consts.tile([128, T], BF16, name=f"mcnat{p}", tag=f"mcnat{p}")
                 for p in range(2)]
        ysb = [consts.tile([128, 128], F32, name=f"ysb{qb}", tag=f"ysb{qb}")
               for qb in range(NB)]

        expool = ctx.enter_context(tc.tile_pool(name="expool", bufs=4))
        srp = ctx.enter_context(tc.tile_pool(name="srp", bufs=2))

        def vsl(b, lo, hi):
            # [v_h | 1/beta] interleaved layout: strided [128, heads, 64]
            return xv[b].rearrange("p (h c) -> p h c", h=HL)[:, lo:hi, 0:64]

        def emit_mc(p, blo, bhi):
            for base in range(blo, bhi, 4):
                mt = mcp.tile([128, 512], F32, name="mct", tag="mcps")
                for bi in range(base, base + 4):
                    sl = mt[:, (bi % 4) * 128:(bi % 4) * 128 + 128]
                    rb = (bi // 6) * 32
                    cc = bi % 6
                    first = True
                    if gamma != 0.0:
                        nc.tensor.matmul(
                            sl,
                            crow[rb:rb + 1, cc * 128:cc * 128 + 128],
                            saft[rb:rb + 1, cc * 256 + p * 128:
                                 cc * 256 + p * 128 + 128],
                            start=True, stop=False)
                        first = False
                    nc.tensor.matmul(sl, mcdTs[:, bi * 128:(bi + 1) * 128],
                                     vsl(bi, 2 * p, 2 * p + 2),
                                     start=first, stop=True)
                    nc.vector.tensor_copy(
                        out=mcnat[p][:, bi * 128:(bi + 1) * 128], in_=sl)

        # projection: one wave = a single 256-query column tt of one qkT
        # tile mt; DoubleRow outs must sit at partition base 0, so the two
        # 64-chan m-halves go to column halves of a [64, 512] bank and the
        # pcopies fan them back out to qkT partitions 0:64 / 64:128.
        proj_ctr = [0]

        def emit_proj(tt, mt):
            proj_ctr[0] += 1
            pool, tg = ((projp, "proj") if proj_ctr[0] % 2
                        else (mcp, "mcps"))
            tp = pool.tile([128, 512], F32, name="prj", tag=tg)
            for mh in (0, 1):
                m = mt * 2 + mh
                out = tp[0:64, mh * 256:mh * 256 + 256]
                mi = WCOL[m]
                for p in range(4):
                    w_ap = wt8s[:, (mi * 4 + p) * 128:
                                (mi * 4 + p) * 128 + 128] \
                        .rearrange("p (two m) -> p two m", two=2)
                    x_ap = xt8s[:, (tt * 4 + p) * 512:
                                (tt * 4 + p) * 512 + 512] \
                        .rearrange("p (two n) -> p two n", two=2)
                    nc.tensor.matmul(
                        out, w_ap, x_ap,
                        start=(p == 0 and mh == 0),
                        stop=(p == 3), perf_mode=DR)
            for mh in (0, 1):
                nc.vector.tensor_copy(
                    out=qkT[mt][mh * 64:mh * 64 + 64,
                                tt * 256:(tt + 1) * 256],
                    in_=tp[0:64, mh * 256:mh * 256 + 256])

        # ---- attention chunk machinery (software-pipelined PV) ----
        # PV groups trail their exp by 2 ST-groups so their Ldweights
        # enter the 4-slot PE wait queue with deps already satisfied.
        pending = []

        def push_group(entry):
            pending.append(entry)
            while len(pending) > 2:
                flush_one()

        def flush_one():
            if not pending:
                return
            h, ct, ya, members = pending.pop(0)
            for kb, ext, colfn, qlo in members:
                for ql in range(qlo, 4):
                    qb = ct * 4 + ql
                    lhsT = ext[:, colfn(ql):colfn(ql) + 128]
                    rhs = xv[kb][:, h * 65:h * 65 + 65]
                    out = ya[:, ql * 65:ql * 65 + 65]
                    nc.tensor.matmul(
                        out, lhsT, rhs,
                        start=(kb == 0 and ql == 0),
                        stop=(kb == qb))
                if kb - ct * 4 >= 0:
                    # diag member: this qb's accumulation just finished
                    emit_combine_q(h, ct, ya, kb - ct * 4)

        def emit_combine_q(h, ct, ya, ql):
            p = h // 2
            qb = ct * 4 + ql
            r1 = srp.tile([128, 1], F32, name="r1", tag="r1")
            nc.vector.reciprocal(out=r1,
                                 in_=ya[:, ql * 65 + 64:ql * 65 + 65])
            outsl = ysb[qb][:, (h % 2) * 64:(h % 2) * 64 + 64]
            nc.vector.scalar_tensor_tensor(
                out=outsl,
                in0=ya[:, ql * 65:ql * 65 + 64],
                scalar=r1,
                in1=mcnat[p][:, qb * 128 + (h % 2) * 64:
                             qb * 128 + (h % 2) * 64 + 64],
                op0=OP.mult, op1=OP.add)
            if h % 2 == 1:
                nc.sync.dma_start(
                    out=y[qb * 128:(qb + 1) * 128,
                          p * 128:(p + 1) * 128],
                    in_=ysb[qb])

        def emit_chunk(h, ct, filler=()):
            pr = (h % 2) * 64
            qT = qkT[h // 2]
            kT = qkT[2 + h // 2]
            c0 = ct * 512
            ya = yaccp.tile([128, 260], F32, name="ya", tag="yacc")
            fill = [list(filler)]

            def run_filler():
                for f in fill[0]:
                    f()
                fill[0] = []

            def st(kb, dst, qlo_col, width, start):
                nc.tensor.matmul(
                    dst,
                    kT[pr:pr + 64, kb * 128:(kb + 1) * 128],
                    qT[pr:pr + 64, c0 + qlo_col:c0 + qlo_col + width],
                    start=start, stop=False)

            def trimask(dst):
                nc.tensor.matmul(dst, negmaskT, identr, start=False,
                                 stop=True)

            # full pairs
            for m in range(ct * 2):
                kA, kB = 2 * m, 2 * m + 1
                tp = bigp.tile([128, 1024], F32, name="stf", tag="big")
                nc.tensor.matmul(
                    tp[:, 0:512], kT[pr:pr + 64, kA * 128:kA * 128 + 128],
                    qT[pr:pr + 64, c0:c0 + 512], start=True, stop=True)
                nc.tensor.matmul(
                    tp[:, 512:1024], kT[pr:pr + 64, kB * 128:kB * 128 + 128],
                    qT[pr:pr + 64, c0:c0 + 512], start=True, stop=True)
                ext = expool.tile([128, 1024], BF16, name="ex", tag="ex")
                nc.scalar.activation(out=ext, in_=tp, func=EXP,
                                     scale=st_scale)
                push_group((h, ct, ya, [
                    (kA, ext, lambda ql: ql * 128, 0),
                    (kB, ext, lambda ql: 512 + ql * 128, 0),
                ]))
                run_filler()

            # diag group 1: k0 (n=512) + k1 (n=384)
            k0, k1 = ct * 4, ct * 4 + 1
            tp = bigp.tile([128, 1024], F32, name="std1", tag="big")
            st(k0, tp[:, 0:512], 0, 512, True)
            trimask(tp[:, 0:128])
            st(k1, tp[:, 512:896], 128, 384, True)
            trimask(tp[:, 512:640])
            ext1 = expool.tile([128, 1024], BF16, name="exd1", tag="ex")
            nc.scalar.activation(out=ext1[:, 0:896], in_=tp[:, 0:896],
                                 func=EXP, scale=st_scale)
            push_group((h, ct, ya, [
                (k0, ext1, lambda ql: ql * 128, 0),
                (k1, ext1, lambda ql: 512 + (ql - 1) * 128, 1),
            ]))
            run_filler()

            # diag group 2: k2 (n=256) + k3 (n=128) in one bank
            k2, k3 = ct * 4 + 2, ct * 4 + 3
            tp2 = bigp.tile([128, 1024], F32, name="std2", tag="big")
            st(k2, tp2[:, 0:256], 256, 256, True)
            trimask(tp2[:, 0:128])
            st(k3, tp2[:, 256:384], 384, 128, False)
            trimask(tp2[:, 256:384])
            ext2 = expool.tile([128, 1024], BF16, name="exd2", tag="ex")
            nc.scalar.activation(out=ext2[:, 0:384], in_=tp2[:, 0:384],
                                 func=EXP, scale=st_scale)
            push_group((h, ct, ya, [
                (k2, ext2, lambda ql: (ql - 2) * 128, 2),
                (k3, ext2, lambda ql: 256, 3),
            ]))
            run_filler()

        # ================= emission order =================
        # 4-head round-robin per chunk; proj/mc filler runs inside the
        # next chunk (after its first exp group) so ACT never drains at
        # chunk boundaries.
        def P(tt, mt):
            return lambda: emit_proj(tt, mt)

        def M(p, lo, hi):
            return lambda: emit_mc(p, lo, hi)

        emit_proj(0, 0)
        emit_proj(0, 2)
        emit_proj(1, 0)
        emit_proj(1, 2)
        emit_chunk(0, 0)
        emit_mc(0, 0, 4)
        emit_proj(0, 1)
        emit_proj(1, 1)
        emit_chunk(1, 0)
        emit_proj(0, 3)
        emit_proj(1, 3)
        emit_chunk(2, 0)
        emit_mc(1, 0, 4)
        emit_proj(2, 0)
        emit_proj(2, 2)
        emit_chunk(3, 0)
        emit_proj(3, 0)
        emit_proj(3, 2)
        emit_chunk(0, 1)
        emit_mc(0, 4, 8)
        emit_proj(2, 1)
        emit_proj(2, 3)
        emit_chunk(1, 1)
        emit_proj(3, 1)
        emit_proj(3, 3)
        emit_chunk(2, 1)
        emit_mc(1, 4, 8)
        emit_proj(4, 0)
        emit_proj(4, 2)
        emit_chunk(3, 1)
        emit_proj(5, 0)
        emit_proj(5, 2)
        emit_chunk(0, 2)
        emit_mc(0, 8, 16)
        emit_proj(4, 1)
        emit_proj(4, 3)
        emit_chunk(1, 2)
        emit_proj(5, 1)
        emit_proj(5, 3)
        emit_chunk(2, 2)
        emit_mc(1, 8, 16)
        emit_proj(6, 0)
        emit_proj(6, 2)
        emit_chunk(3, 2)
        emit_proj(7, 0)
        emit_proj(7, 2)
        emit_chunk(0, 3)
        emit_proj(6, 1)
        emit_proj(6, 3)
        emit_chunk(1, 3)
        emit_proj(7, 1)
        emit_proj(7, 3)
        emit_chunk(2, 3)
        emit_chunk(3, 3)
        flush_one()
        flush_one()


_BUILD_CACHE = {}


def build_nc(alpha, beta, gamma, cfg=None):
    cfg = dict(CFG if cfg is None else cfg)
    key = (alpha, beta, gamma, tuple(sorted(cfg.items())))
    if key in _BUILD_CACHE:
        return _BUILD_CACHE[key]
    nc = bacc.Bacc("TRN2", target_bir_lowering=False, debug=False,
                   num_devices=NCORES)
    xt8 = nc.dram_tensor("xt8", [128, 8 * 2048], F8, kind="ExternalInput").ap()
    wt8 = nc.dram_tensor("wt8", [128, 4096], F8, kind="ExternalInput").ap()
    xv16 = nc.dram_tensor("xv16", [T, HL * 65], BF16,
                          kind="ExternalInput").ap()
    mcdTd = nc.dram_tensor("mcdTd", [128, NB * 128], BF16,
                           kind="ExternalInput").ap()
    crowd = nc.dram_tensor("crowd", [128, 768], BF16,
                           kind="ExternalInput").ap()
    safterd = nc.dram_tensor("safterd", [128, 1536], BF16,
                             kind="ExternalInput").ap()
    y = nc.dram_tensor("y", [T, GC], F32, kind="ExternalOutput").ap()
    with tile.TileContext(nc) as tc:
        _emit(tc, xt8, wt8, xv16, mcdTd, crowd, safterd, y,
              alpha, beta, gamma, cfg)
    nc.compile()
    _BUILD_CACHE[key] = nc
    return nc


def make_in_maps(x, w, alpha, beta, gamma):
    # xt8 per batch: [128, 16384]; col (t*4+p)*512 + half*256 + tc holds
    # x[b, t*256+tc, p*256 + half*128 + r]^T * SX
    xt8s = []
    for b in range(B):
        xbT = np.ascontiguousarray(x[b].T)            # [C, T]
        arr = (xbT * SX).reshape(4, 2, 128, 8, 256)   # p, half, r, t, col
        arr = arr.transpose(2, 3, 0, 1, 4).reshape(128, 8 * 2048)
        xt8s.append(np.ascontiguousarray(arr).astype(NPF8))

    inv_beta = 1.0 / beta if beta != 0.0 else 1.0

    # MC constants (same for all cores): c(gi) = -gamma/(T-1-gi), c(T-1)=0
    gi = np.arange(T, dtype=np.float64)
    c = np.zeros(T, np.float64)
    c[:-1] = -gamma / (T - 1 - gi[:-1])
    crow = c.astype(np.float32)
    crow15 = np.zeros(128, np.float32)
    crow15[127] = -gamma / T
    il = np.arange(128)
    upper = (il[:, None] > il[None, :]).astype(np.float32)  # [j, i] j > i
    eye = np.eye(128, dtype=np.float32)
    mcdT = np.empty((128, NB * 128), np.float32)
    for bi in range(NB):
        mcdT[:, bi * 128:(bi + 1) * 128] = \
            upper * crow[bi * 128:(bi + 1) * 128][None, :] + alpha * eye
    mcdT16 = mcdT.astype(NPBF)
    crowd = np.zeros((128, 768), np.float32)
    for bi in range(NB):
        rb, cc = (bi // 6) * 32, bi % 6
        src = crow15 if bi == NB - 1 else crow[bi * 128:(bi + 1) * 128]
        crowd[rb, cc * 128:cc * 128 + 128] = src
    crowd16 = crowd.astype(NPBF)

    in_maps = []
    for cidx in range(NCORES):
        b, g = cidx // HL, cidx % HL
        wqk = np.concatenate(
            [w[GC * g:GC * (g + 1)], w[C + GC * g:C + GC * (g + 1)]], axis=0)
        wTs = (wqk.T * SW).reshape(4, 2, 128, 8, 64)   # p, half, r, m, mc
        wt8f = wTs.transpose(2, 3, 0, 1, 4)            # r, m, p, half, mc
        morder = [0, 1, 4, 5, 2, 3, 6, 7]
        wt8f = wt8f[:, morder]
        wt8 = np.ascontiguousarray(wt8f.reshape(128, 4096)).astype(NPF8)

        xv = np.empty((T, HL * 65), np.float32)
        for h in range(HL):
            xv[:, h * 65:h * 65 + 64] = x[b][:, GC * g + 64 * h:
                                             GC * g + 64 * h + 64]
            xv[:, h * 65 + 64] = inv_beta

        # suffix block sums of v: safter[bi] = sum of v rows in blocks > bi
        # (block 15 slot holds the TOTAL row sum, for the dense last MC row)
        v = x[b][:, GC * g:GC * (g + 1)].astype(np.float64)
        bsum = v.reshape(NB, 128, GC).sum(axis=1)          # [16, 256]
        suff = np.zeros((NB, GC), np.float64)
        suff[:-1] = bsum[::-1].cumsum(axis=0)[::-1][1:]
        suff[NB - 1] = bsum.sum(axis=0)
        saftd = np.zeros((128, 1536), np.float32)
        for bi in range(NB):
            saftd[(bi // 6) * 32, (bi % 6) * 256:(bi % 6) * 256 + 256] = \
                suff[bi]

        in_maps.append({
            "xt8": xt8s[b],
            "wt8": wt8,
            "xv16": xv.astype(NPBF),
            "mcdTd": mcdT16,
            "crowd": crowd16,
            "safterd": saftd.astype(NPBF),
        })
    return in_maps


def kernel(x, w_attn, alpha, beta, gamma, n_head, **run_kwargs):
    global LAST_RESULTS
    x = np.asarray(x, dtype=np.float32)
    w = np.asarray(w_attn, dtype=np.float32)
    assert int(n_head) == H and x.shape == (B, T, C)
    nc = build_nc(float(alpha), float(beta), float(gamma))
    res = run_bass_kernel_spmd(
        nc, make_in_maps(x, w, float(alpha), float(beta), float(gamma)),
        list(range(NCORES)), **run_kwargs)
    LAST_RESULTS = res
    out = np.empty((B, T, C), dtype=np.float32)
    for cidx in range(NCORES):
        b, g = cidx // HL, cidx % HL
        out[b][:, GC * g:GC * (g + 1)] = res.results[cidx]["y"]
    return out
